# revision 20
# baseline (speedup 1.0000x reference)
"""Multi-head attention (unstabilized softmax) on 8 TRN2 NeuronCores.

Reference computes, per (batch, head):
    scores  = Q @ K^T / sqrt(d)          [S, S]
    weights = exp(scores) / rowsum(exp)  (unstabilized softmax)
    out     = weights @ V                [S, d]

Sharding: B*H = 64 (batch, head) pairs split across 8 cores -> 8 heads per
core, fully independent (no collectives).

Device pipeline per head (S=2048, d=128), q-chunk-major.  ACT (exp at
1 elem/lane/cycle @1.2GHz, ~185ns fixed overhead per instruction) is the
critical engine; everything else hides in its shadow:

  Per q-chunk qc (512 cols) and k-tile kt (128 rows), mm1 computes
  scoresT[kt, qc] = K_chunk^T.T @ Q^T[:, qc] into 2 ping-ponged PSUM slots
  of [128, 3, 512]; ACT exp consumes (3,3,3,3,2,2) blocks per instruction
  (N=1536/1024) -> bf16 W^T chunks in SBUF.  mm2 (PE filler work,
  scheduled by a PE/ACT clock model so every exp's mm1 lands with margin):
  per 128-row q-tile, accumulate over kt:
      po[q, 0:129] += W^T[kt,q].T @ [V|1][kt]      (ones col -> rowsum)
  epilogue: DVE reciprocal + per-partition multiply into a per-head output
  buffer; one 1MB store per head (DMA and teardown semaphores are
  per-dma_start; few big transfers beat many 64KB ones).

Host prep: per-head inputs packed partition-major into one DRAM tensor
(K^T | Q^T | V-augmented) in bf16, so each head loads with a single
contiguous-per-partition DMA and no device-side casts.
"""

import collections
import math
import os

import numpy as np
import ml_dtypes

import concourse.bass as bass  # noqa: F401  (bass types used via APs)
import concourse.mybir as mybir
from concourse import bacc
from concourse.tile import TileContext
from concourse.bass_utils import run_bass_kernel_spmd

B, H, S, D = 4, 16, 2048, 128
N_CORES = 8
HPC = (B * H) // N_CORES  # heads per core
SCALE = 1.0 / math.sqrt(D)

KT = S // 128          # 16 k-tiles per head
QC = S // 512          # 4 q-chunks per head
BLK = 512              # score block = one mm1 matmul (N=512, one PSUM bank)

# Packed per-head input layout (bf16 elements per partition):
#   [ K^T row | Q^T row | Va row(16 x 129) ]
K_OFF = 0
Q_OFF = S
VA_OFF = 2 * S
PACK_W = 2 * S + KT * (D + 1)

# PE/ACT cost model for the filler scheduler (ns)
MM1_NS = 216.0
MM2_NS = 59.0
EXP_OVH = 151.0
ACT_CYC = 1 / 1.2
# Reserving headroom before each exp costs filler capacity: total PE work
# (~7.23us/phase) vs ACT (~7.73us/phase) leaves only ~500ns/phase of true
# slack, so the per-unit margin must stay small or the filler queue starves.
SEM_MARGIN = 70.0

LAST_EXEC_TIME_NS = None
LAST_RESULTS = None
_NC_CACHE = {}


def build(hpc=HPC):
    f32 = mybir.dt.float32
    bf16 = mybir.dt.bfloat16

    nc = bacc.Bacc(None, target_bir_lowering=False)

    qkv_d = nc.declare_dram_parameter("qkv", [hpc, 128, PACK_W], bf16, isOutput=False)
    # Output partition-major [h, p, qt, d]: contiguous per-partition runs.
    o_d = nc.declare_dram_parameter("out", [hpc, 128, KT, D], f32, isOutput=True)

    # exp stitch plan per phase: (kt_start, n_blocks).  Trailing units are
    # 2-blocks each (not 3+1): every exp must outlast the next unit's mm1
    # refill or ACT bubbles at the phase seam.
    plan = [(0, 3), (3, 3), (6, 3), (9, 3), (12, 2), (14, 2)]

    with TileContext(nc) as tc:
        with (
            tc.tile_pool(name="qkv", bufs=2) as qkv_pool,
            tc.tile_pool(name="wt", bufs=3) as wt_pool,
            tc.tile_pool(name="osb", bufs=2) as osb_pool,
            tc.tile_pool(name="rc", bufs=4) as rc_pool,
            tc.tile_pool(name="scoreps", bufs=2, space="PSUM") as score_pool,
            tc.tile_pool(name="outps", bufs=2, space="PSUM") as out_ps_pool,
        ):
            head_state = {}

            def load_head(h):
                """One packed DMA per head (head 0: split for startup)."""
                qkv_sb = qkv_pool.tile([128, PACK_W], bf16, tag="qkv")
                if h == 0:
                    # The first exp needs K cols 0:512 and Q cols 0:512;
                    # issue those on separate DGE rings (each dma_start is
                    # ~600ns of serial sequencer issue), then the rest.
                    nc.sync.dma_start(
                        out=qkv_sb[:, K_OFF : K_OFF + 512],
                        in_=qkv_d[h, :, K_OFF : K_OFF + 512],
                    )
                    nc.gpsimd.dma_start(
                        out=qkv_sb[:, Q_OFF : Q_OFF + 512],
                        in_=qkv_d[h, :, Q_OFF : Q_OFF + 512],
                    )
                    nc.sync.dma_start(
                        out=qkv_sb[:, K_OFF + 512 : Q_OFF],
                        in_=qkv_d[h, :, K_OFF + 512 : Q_OFF],
                    )
                    nc.gpsimd.dma_start(
                        out=qkv_sb[:, Q_OFF + 512 : PACK_W],
                        in_=qkv_d[h, :, Q_OFF + 512 : PACK_W],
                    )
                else:
                    eng = nc.gpsimd if h % 2 else nc.sync
                    eng.dma_start(out=qkv_sb, in_=qkv_d[h])
                k_sb = qkv_sb[:, K_OFF : K_OFF + S]
                q_sb = qkv_sb[:, Q_OFF : Q_OFF + S]
                va_sb = qkv_sb[:, VA_OFF:PACK_W].rearrange(
                    "p (kt d) -> p kt d", kt=KT
                )
                return q_sb, k_sb, va_sb

            def mm2_closures(h, qc, wt):
                """(cost_ns, closure) list: 64 mm2 matmuls + 4 epilogues."""
                va_sb = head_state[h][2]
                shared = {}
                out = []
                for qi in range(4):
                    po_box = {}

                    def mk_mm(kt, qi=qi, po_box=po_box):
                        def go():
                            if kt == 0:
                                po = out_ps_pool.tile([128, D + 1], f32, tag="po")
                                po_box["po"] = po
                            nc.tensor.matmul(
                                out=po_box["po"],
                                lhsT=wt[:, kt, qi * 128 : (qi + 1) * 128],
                                rhs=va_sb[:, kt, :],
                                start=(kt == 0),
                                stop=(kt == KT - 1),
                            )
                        return go

                    last_head = h == hpc - 1

                    def mk_epi(qi=qi, qc=qc, po_box=po_box, last_head=last_head):
                        def go():
                            po = po_box["po"]
                            if qc == 0 and qi == 0:
                                o_sb = osb_pool.tile([128, KT, D], f32, tag="osb")
                                shared["o_sb"] = o_sb
                                head_state[h] = head_state[h][:3] + (o_sb,)
                            o_sb = head_state[h][3] if qc > 0 else shared["o_sb"]
                            qg = qc * 4 + qi
                            rc = rc_pool.tile([128, 1], f32, tag="rc")
                            nc.vector.reciprocal(out=rc, in_=po[:, D : D + 1])
                            nc.vector.tensor_scalar_mul(
                                o_sb[:, qg, :], po[:, 0:D], rc
                            )
                            if last_head and qc == QC - 1:
                                # Tail: ship each q-tile as it completes; the
                                # store completion wait is on the critical
                                # path.
                                nc.sync.dma_start(
                                    out=o_d[h, :, qg, :], in_=o_sb[:, qg, :]
                                )
                            elif last_head and qc == QC - 2 and qi == 3:
                                nc.sync.dma_start(
                                    out=o_d[h, :, 0 : 3 * 4, :],
                                    in_=o_sb[:, 0 : 3 * 4, :],
                                )
                            elif qc == QC - 1 and qi == 3:
                                # One 1MB store per head.
                                eng = nc.gpsimd if h % 2 else nc.sync
                                eng.dma_start(out=o_d[h], in_=o_sb)
                        return go

                    for kt in range(KT):
                        out.append((MM2_NS, mk_mm(kt)))
                    out.append((0.0, mk_epi()))
                return out

            # Filler scheduler state: pe_t = projected PE-busy frontier,
            # act_t = projected ACT completion frontier.
            fillers = collections.deque()
            clock = {"pe": 0.0, "act": 0.0}

            def emit_phase(h, qc):
                if (h, qc) == (0, 0):
                    head_state[0] = load_head(0)
                if qc == QC - 1 and h + 1 < hpc:
                    head_state[h + 1] = load_head(h + 1)
                q_sb, k_sb, _ = head_state[h][:3]
                wt = wt_pool.tile([128, KT, 512], bf16, tag="wt")
                q0 = qc * 512

                for kt0, nb in plan:
                    mm1_cost = nb * MM1_NS
                    # Drain fillers while the PE can still deliver this
                    # unit's mm1 before the previous exp completes.
                    while fillers:
                        c, go = fillers[0]
                        if clock["pe"] + c + mm1_cost + SEM_MARGIN > clock["act"]:
                            break
                        fillers.popleft()
                        go()
                        clock["pe"] += c
                    ps = score_pool.tile([128, 3, BLK], f32, tag="score")
                    for j in range(nb):
                        kt = kt0 + j
                        nc.tensor.matmul(
                            out=ps[:, j, :],
                            lhsT=k_sb[:, kt * 128 : (kt + 1) * 128],
                            rhs=q_sb[:, q0 : q0 + 512],
                            start=True,
                            stop=True,
                        )
                    clock["pe"] += mm1_cost
                    nc.scalar.activation(
                        out=wt[:, kt0 : kt0 + nb, :],
                        in_=ps[:, 0:nb, :],
                        func=mybir.ActivationFunctionType.Exp,
                        scale=SCALE,
                    )
                    exp_dur = nb * BLK * ACT_CYC + EXP_OVH
                    clock["act"] = (
                        max(clock["act"], clock["pe"] + SEM_MARGIN) + exp_dur
                    )
                return wt

            prev = None
            for h in range(hpc):
                for qc in range(QC):
                    if prev is not None:
                        fillers.extend(mm2_closures(*prev))
                        assert len(fillers) <= 140, len(fillers)
                    wt = emit_phase(h, qc)
                    prev = (h, qc, wt)
            fillers.extend(mm2_closures(*prev))
            for _, go in fillers:
                go()

    return nc


def _shard_host(Q, K, V, hpc, n_cores):
    """Host-side shard + pack + cast: returns per-core input maps."""
    bf16 = ml_dtypes.bfloat16
    BH = Q.shape[0] * Q.shape[1]
    s, d = Q.shape[2], Q.shape[3]
    kt_n = s // 128
    Qf = Q.reshape(BH, s, d)
    Kf = K.reshape(BH, s, d)
    Vf = V.reshape(BH, s, d)
    pack = np.empty((BH, 128, PACK_W), dtype=bf16)
    pack[:, :, K_OFF : K_OFF + s] = Kf.transpose(0, 2, 1).astype(bf16)
    pack[:, :, Q_OFF : Q_OFF + s] = Qf.transpose(0, 2, 1).astype(bf16)
    va = pack[:, :, VA_OFF:].reshape(BH, 128, kt_n, d + 1)
    va[:, :, :, 0:d] = Vf.reshape(BH, kt_n, 128, d).transpose(0, 2, 1, 3).astype(bf16)
    va[:, :, :, d] = 1.0
    return [
        {"qkv": pack[c * hpc : (c + 1) * hpc]} for c in range(n_cores)
    ]


def kernel(Q, K, V):
    global LAST_EXEC_TIME_NS, LAST_RESULTS
    Q = np.asarray(Q, dtype=np.float32)
    K = np.asarray(K, dtype=np.float32)
    V = np.asarray(V, dtype=np.float32)

    trace = os.environ.get("ATTN_TRACE", "0") == "1"

    key = (HPC, S)
    nc = _NC_CACHE.get(key)
    if nc is None:
        nc = build(hpc=HPC)
        nc.compile()
        _NC_CACHE[key] = nc

    in_maps = _shard_host(Q, K, V, HPC, N_CORES)
    res = run_bass_kernel_spmd(nc, in_maps, core_ids=list(range(N_CORES)), trace=trace)
    LAST_EXEC_TIME_NS = res.exec_time_ns
    LAST_RESULTS = res

    # Device out layout is partition-major [hpc, p, qt, d] -> [hpc, S, D].
    out = np.concatenate([res.results[c]["out"] for c in range(N_CORES)], axis=0)
    out = out.reshape(B * H, 128, KT, D).transpose(0, 2, 1, 3)
    return np.ascontiguousarray(out.reshape(B, H, S, D))


# revision 21
# speedup vs baseline: 1.0032x; 1.0032x over previous
"""Multi-head attention (unstabilized softmax) on 8 TRN2 NeuronCores.

Reference computes, per (batch, head):
    scores  = Q @ K^T / sqrt(d)          [S, S]
    weights = exp(scores) / rowsum(exp)  (unstabilized softmax)
    out     = weights @ V                [S, d]

Sharding: B*H = 64 (batch, head) pairs split across 8 cores -> 8 heads per
core, fully independent (no collectives).

Device pipeline per head (S=2048, d=128), q-chunk-major.  ACT (exp at
1 elem/lane/cycle @1.2GHz, ~185ns fixed overhead per instruction) is the
critical engine; everything else hides in its shadow:

  Per q-chunk qc (512 cols) and k-tile kt (128 rows), mm1 computes
  scoresT[kt, qc] = K_chunk^T.T @ Q^T[:, qc] into 2 ping-ponged PSUM slots
  of [128, 3, 512]; ACT exp consumes (3,3,3,3,2,2) blocks per instruction
  (N=1536/1024) -> bf16 W^T chunks in SBUF.  mm2 (PE filler work,
  scheduled by a PE/ACT clock model so every exp's mm1 lands with margin):
  per 128-row q-tile, accumulate over kt:
      po[q, 0:129] += W^T[kt,q].T @ [V|1][kt]      (ones col -> rowsum)
  epilogue: DVE reciprocal + per-partition multiply into a per-head output
  buffer; one 1MB store per head (DMA and teardown semaphores are
  per-dma_start; few big transfers beat many 64KB ones).

Host prep: per-head inputs packed partition-major into one DRAM tensor
(K^T | Q^T | V-augmented) in bf16, so each head loads with a single
contiguous-per-partition DMA and no device-side casts.
"""

import collections
import math
import os

import numpy as np
import ml_dtypes

import concourse.bass as bass  # noqa: F401  (bass types used via APs)
import concourse.mybir as mybir
from concourse import bacc
from concourse.tile import TileContext
from concourse.bass_utils import run_bass_kernel_spmd

B, H, S, D = 4, 16, 2048, 128
N_CORES = 8
HPC = (B * H) // N_CORES  # heads per core
SCALE = 1.0 / math.sqrt(D)

KT = S // 128          # 16 k-tiles per head
QC = S // 512          # 4 q-chunks per head
BLK = 512              # score block = one mm1 matmul (N=512, one PSUM bank)

# Packed per-head input layout (bf16 elements per partition):
#   [ K^T row | Q^T row | Va row(16 x 129) ]
K_OFF = 0
Q_OFF = S
VA_OFF = 2 * S
PACK_W = 2 * S + KT * (D + 1)

# PE/ACT cost model for the filler scheduler (ns)
MM1_NS = 216.0
MM2_NS = 59.0
EXP_OVH = 151.0
ACT_CYC = 1 / 1.2
# Reserving headroom before each exp costs filler capacity: total PE work
# (~7.23us/phase) vs ACT (~7.73us/phase) leaves only ~500ns/phase of true
# slack, so the per-unit margin must stay small or the filler queue starves.
SEM_MARGIN = 70.0

LAST_EXEC_TIME_NS = None
LAST_RESULTS = None
_NC_CACHE = {}


def build(hpc=HPC):
    f32 = mybir.dt.float32
    bf16 = mybir.dt.bfloat16

    nc = bacc.Bacc(None, target_bir_lowering=False)

    qkv_d = nc.declare_dram_parameter("qkv", [hpc, 128, PACK_W], bf16, isOutput=False)
    # Output partition-major [h, p, qt, d]: contiguous per-partition runs.
    o_d = nc.declare_dram_parameter("out", [hpc, 128, KT, D], f32, isOutput=True)

    # exp stitch plan per phase: (kt_start, n_blocks).  Trailing units are
    # 2-blocks each (not 3+1): every exp must outlast the next unit's mm1
    # refill or ACT bubbles at the phase seam.
    plan = [(0, 3), (3, 3), (6, 3), (9, 3), (12, 2), (14, 2)]

    with TileContext(nc) as tc:
        with (
            tc.tile_pool(name="qkv", bufs=2) as qkv_pool,
            tc.tile_pool(name="wt", bufs=3) as wt_pool,
            tc.tile_pool(name="osb", bufs=2) as osb_pool,
            tc.tile_pool(name="rc", bufs=4) as rc_pool,
            tc.tile_pool(name="scoreps", bufs=2, space="PSUM") as score_pool,
            tc.tile_pool(name="outps", bufs=2, space="PSUM") as out_ps_pool,
        ):
            head_state = {}

            def load_head(h):
                """One packed DMA per head (head 0: split for startup)."""
                qkv_sb = qkv_pool.tile([128, PACK_W], bf16, tag="qkv")
                if h == 0:
                    # The first exp needs K cols 0:512 and Q cols 0:512;
                    # issue those on separate DGE rings (each dma_start is
                    # ~600ns of serial sequencer issue), then the rest.
                    nc.sync.dma_start(
                        out=qkv_sb[:, K_OFF : K_OFF + 512],
                        in_=qkv_d[h, :, K_OFF : K_OFF + 512],
                    )
                    nc.gpsimd.dma_start(
                        out=qkv_sb[:, Q_OFF : Q_OFF + 512],
                        in_=qkv_d[h, :, Q_OFF : Q_OFF + 512],
                    )
                    nc.sync.dma_start(
                        out=qkv_sb[:, K_OFF + 512 : Q_OFF],
                        in_=qkv_d[h, :, K_OFF + 512 : Q_OFF],
                    )
                    nc.gpsimd.dma_start(
                        out=qkv_sb[:, Q_OFF + 512 : PACK_W],
                        in_=qkv_d[h, :, Q_OFF + 512 : PACK_W],
                    )
                else:
                    eng = nc.gpsimd if h % 2 else nc.sync
                    eng.dma_start(out=qkv_sb, in_=qkv_d[h])
                k_sb = qkv_sb[:, K_OFF : K_OFF + S]
                q_sb = qkv_sb[:, Q_OFF : Q_OFF + S]
                va_sb = qkv_sb[:, VA_OFF:PACK_W].rearrange(
                    "p (kt d) -> p kt d", kt=KT
                )
                return q_sb, k_sb, va_sb

            def mm2_closures(h, qc, wt):
                """(cost_ns, closure) list: 64 mm2 matmuls + 4 epilogues."""
                va_sb = head_state[h][2]
                shared = {}
                out = []
                for qi in range(4):
                    po_box = {}

                    def mk_mm(kt, qi=qi, po_box=po_box):
                        def go():
                            if kt == 0:
                                po = out_ps_pool.tile([128, D + 1], f32, tag="po")
                                po_box["po"] = po
                            nc.tensor.matmul(
                                out=po_box["po"],
                                lhsT=wt[:, kt, qi * 128 : (qi + 1) * 128],
                                rhs=va_sb[:, kt, :],
                                start=(kt == 0),
                                stop=(kt == KT - 1),
                            )
                        return go

                    last_head = h == hpc - 1

                    def mk_epi(qi=qi, qc=qc, po_box=po_box, last_head=last_head):
                        def go():
                            po = po_box["po"]
                            if qc == 0 and qi == 0:
                                o_sb = osb_pool.tile([128, KT, D], f32, tag="osb")
                                shared["o_sb"] = o_sb
                                head_state[h] = head_state[h][:3] + (o_sb,)
                            o_sb = head_state[h][3] if qc > 0 else shared["o_sb"]
                            qg = qc * 4 + qi
                            rc = rc_pool.tile([128, 1], f32, tag="rc")
                            nc.vector.reciprocal(out=rc, in_=po[:, D : D + 1])
                            nc.vector.tensor_scalar_mul(
                                o_sb[:, qg, :], po[:, 0:D], rc
                            )
                            if last_head and qc == QC - 1:
                                # Tail: ship each q-tile as it completes; the
                                # store completion wait is on the critical
                                # path.
                                nc.sync.dma_start(
                                    out=o_d[h, :, qg, :], in_=o_sb[:, qg, :]
                                )
                            elif last_head and qc == QC - 2 and qi == 3:
                                nc.sync.dma_start(
                                    out=o_d[h, :, 0 : 3 * 4, :],
                                    in_=o_sb[:, 0 : 3 * 4, :],
                                )
                            elif qc == QC - 1 and qi == 3:
                                # One 1MB store per head.
                                eng = nc.gpsimd if h % 2 else nc.sync
                                eng.dma_start(out=o_d[h], in_=o_sb)
                        return go

                    for kt in range(KT):
                        out.append((MM2_NS, mk_mm(kt)))
                    out.append((0.0, mk_epi()))
                return out

            # Filler scheduler state: pe_t = projected PE-busy frontier,
            # act_t = projected ACT completion frontier.
            fillers = collections.deque()
            clock = {"pe": 0.0, "act": 0.0}

            def emit_phase(h, qc):
                if (h, qc) == (0, 0):
                    head_state[0] = load_head(0)
                if qc == QC - 1 and h + 1 < hpc:
                    head_state[h + 1] = load_head(h + 1)
                q_sb, k_sb, _ = head_state[h][:3]
                wt = wt_pool.tile([128, KT, 512], bf16, tag="wt")
                q0 = qc * 512

                for kt0, nb in plan:
                    mm1_cost = nb * MM1_NS
                    # Drain fillers while the PE can still deliver this
                    # unit's mm1 before the previous exp completes.  The
                    # modeled ACT lead is clamped: the real slack per phase
                    # is small, and an unclamped lead drains a whole phase
                    # of fillers as one burst at the seam, which stalls on
                    # the just-finished exp's wt and gaps ACT.
                    budget = min(clock["act"] - clock["pe"], 2200.0)
                    budget -= mm1_cost + SEM_MARGIN
                    while fillers:
                        c, go = fillers[0]
                        if c > budget:
                            break
                        fillers.popleft()
                        go()
                        clock["pe"] += c
                        budget -= c
                    ps = score_pool.tile([128, 3, BLK], f32, tag="score")
                    for j in range(nb):
                        kt = kt0 + j
                        nc.tensor.matmul(
                            out=ps[:, j, :],
                            lhsT=k_sb[:, kt * 128 : (kt + 1) * 128],
                            rhs=q_sb[:, q0 : q0 + 512],
                            start=True,
                            stop=True,
                        )
                    clock["pe"] += mm1_cost
                    nc.scalar.activation(
                        out=wt[:, kt0 : kt0 + nb, :],
                        in_=ps[:, 0:nb, :],
                        func=mybir.ActivationFunctionType.Exp,
                        scale=SCALE,
                    )
                    exp_dur = nb * BLK * ACT_CYC + EXP_OVH
                    clock["act"] = (
                        max(clock["act"], clock["pe"] + SEM_MARGIN) + exp_dur
                    )
                return wt

            prev = None
            for h in range(hpc):
                for qc in range(QC):
                    if prev is not None:
                        fillers.extend(mm2_closures(*prev))
                        assert len(fillers) <= 140, len(fillers)
                    wt = emit_phase(h, qc)
                    prev = (h, qc, wt)
            fillers.extend(mm2_closures(*prev))
            for _, go in fillers:
                go()

    return nc


def _shard_host(Q, K, V, hpc, n_cores):
    """Host-side shard + pack + cast: returns per-core input maps."""
    bf16 = ml_dtypes.bfloat16
    BH = Q.shape[0] * Q.shape[1]
    s, d = Q.shape[2], Q.shape[3]
    kt_n = s // 128
    Qf = Q.reshape(BH, s, d)
    Kf = K.reshape(BH, s, d)
    Vf = V.reshape(BH, s, d)
    pack = np.empty((BH, 128, PACK_W), dtype=bf16)
    pack[:, :, K_OFF : K_OFF + s] = Kf.transpose(0, 2, 1).astype(bf16)
    pack[:, :, Q_OFF : Q_OFF + s] = Qf.transpose(0, 2, 1).astype(bf16)
    va = pack[:, :, VA_OFF:].reshape(BH, 128, kt_n, d + 1)
    va[:, :, :, 0:d] = Vf.reshape(BH, kt_n, 128, d).transpose(0, 2, 1, 3).astype(bf16)
    va[:, :, :, d] = 1.0
    return [
        {"qkv": pack[c * hpc : (c + 1) * hpc]} for c in range(n_cores)
    ]


def kernel(Q, K, V):
    global LAST_EXEC_TIME_NS, LAST_RESULTS
    Q = np.asarray(Q, dtype=np.float32)
    K = np.asarray(K, dtype=np.float32)
    V = np.asarray(V, dtype=np.float32)

    trace = os.environ.get("ATTN_TRACE", "0") == "1"

    key = (HPC, S)
    nc = _NC_CACHE.get(key)
    if nc is None:
        nc = build(hpc=HPC)
        nc.compile()
        _NC_CACHE[key] = nc

    in_maps = _shard_host(Q, K, V, HPC, N_CORES)
    res = run_bass_kernel_spmd(nc, in_maps, core_ids=list(range(N_CORES)), trace=trace)
    LAST_EXEC_TIME_NS = res.exec_time_ns
    LAST_RESULTS = res

    # Device out layout is partition-major [hpc, p, qt, d] -> [hpc, S, D].
    out = np.concatenate([res.results[c]["out"] for c in range(N_CORES)], axis=0)
    out = out.reshape(B * H, 128, KT, D).transpose(0, 2, 1, 3)
    return np.ascontiguousarray(out.reshape(B, H, S, D))


# revision 22
# speedup vs baseline: 1.0663x; 1.0629x over previous
"""Multi-head attention (unstabilized softmax) on 8 TRN2 NeuronCores.

Reference computes, per (batch, head):
    scores  = Q @ K^T / sqrt(d)          [S, S]
    weights = exp(scores) / rowsum(exp)  (unstabilized softmax)
    out     = weights @ V                [S, d]

Sharding: B*H = 64 (batch, head) pairs split across 8 cores -> 8 heads per
core, fully independent (no collectives).

Device pipeline per head (S=2048, d=128), q-chunk-major:
  For each q-chunk qc (512 q columns), for each k-tile kt (128 rows):
    mm1 block: scoresT[kt, qc] = K_chunk^T.T @ Q^T[:, qc]  -> PSUM [128, 512]
  Blocks land in 2 ping-ponged PSUM slots of [128, 3, 512] (3 banks each);
  ACT exp consumes 3 blocks per instruction (N=1536; plus one N=512
  remainder per phase) -> bf16 W^T chunks in SBUF.  Larger ACT tiles
  amortize the ~180-cycle per-instruction overhead (ACT is the critical
  engine: exp throughput is 1 elem/lane/cycle @ 1.2 GHz).
  mm2 (interleaved as PE filler): per 128-row q-tile, accumulate over kt:
    po[q,0:129] += W^T[kt,q].T @ [V|1][kt]   (ones col -> rowsum)
  epilogue: DVE reciprocal of col 128, per-partition scalar multiply,
  DMA out f32.  mm2 for q-chunk qc runs during the exp phase of qc+1, so
  the pipeline tail is only the final q-chunk's mm2 (~4us vs ~19us for
  head-major ordering).

Host prep: Q^T/K^T layouts [d, S] in bf16, V augmented with a ones column
([S, 129] bf16) so no device-side casts are needed.
"""

import math
import os

import numpy as np
import ml_dtypes

import concourse.bass as bass  # noqa: F401  (bass types used via APs)
import concourse.mybir as mybir
from concourse import bacc
from concourse.tile import TileContext
from concourse.bass_utils import run_bass_kernel_spmd

B, H, S, D = 4, 16, 2048, 128
N_CORES = 8
HPC = (B * H) // N_CORES  # heads per core
SCALE = 1.0 / math.sqrt(D)

KT = S // 128          # 16 k-tiles per head
QC = S // 512          # 4 q-chunks per head
BLK = 512              # score block = one mm1 matmul (N=512, one PSUM bank)
STITCH = 3             # blocks per exp instruction (3 banks per PSUM slot)

LAST_EXEC_TIME_NS = None
LAST_RESULTS = None
_NC_CACHE = {}


def build(hpc=HPC):
    f32 = mybir.dt.float32
    bf16 = mybir.dt.bfloat16

    nc = bacc.Bacc(None, target_bir_lowering=False)

    # va/out use partition-major DRAM layouts so each partition's DMA run is
    # contiguous (big descriptors; <64KB strided transfers are
    # descriptor-dominated on the SDMA engines).
    qt_d = nc.declare_dram_parameter("qt", [hpc, D, S], bf16, isOutput=False)
    kt_d = nc.declare_dram_parameter("kt", [hpc, D, S], bf16, isOutput=False)
    va_d = nc.declare_dram_parameter("va", [hpc, 128, KT, D + 1], bf16, isOutput=False)
    o_d = nc.declare_dram_parameter("out", [hpc, 128, KT, D], f32, isOutput=True)

    # exp stitch plan per phase: (kt_start, n_blocks).  The trailing units
    # are 2-blocks each (not 3+1): every exp must be longer than the next
    # unit's mm1 refill (3 blocks = ~650ns) or ACT bubbles at the phase seam.
    plan = [(0, 3), (3, 3), (6, 3), (9, 3), (12, 2), (14, 2)]
    quotas = [13, 13, 13, 13, 9, 7]

    with TileContext(nc) as tc:
        with (
            tc.tile_pool(name="qk", bufs=2) as qk_pool,
            tc.tile_pool(name="va", bufs=2) as va_pool,
            tc.tile_pool(name="wt", bufs=3) as wt_pool,
            tc.tile_pool(name="osb", bufs=4) as osb_pool,
            tc.tile_pool(name="rc", bufs=4) as rc_pool,
            tc.tile_pool(name="scoreps", bufs=2, space="PSUM") as score_pool,
            tc.tile_pool(name="outps", bufs=2, space="PSUM") as out_ps_pool,
        ):
            head_state = {}

            def load_head(h):
                """DMA head h inputs (bf16, no casts needed)."""
                q_sb = qk_pool.tile([128, S], bf16, tag="q")
                k_sb = qk_pool.tile([128, S], bf16, tag="k")
                if h == 0:
                    # Chunk finest: the first exp needs only K cols 0:384
                    # and Q cols 0:512; don't gate on the full tensors.
                    for ci in range(4):
                        c0 = ci * 512
                        nc.sync.dma_start(
                            out=k_sb[:, c0 : c0 + 512], in_=kt_d[h, :, c0 : c0 + 512]
                        )
                        nc.sync.dma_start(
                            out=q_sb[:, c0 : c0 + 512], in_=qt_d[h, :, c0 : c0 + 512]
                        )
                else:
                    nc.sync.dma_start(out=k_sb, in_=kt_d[h])
                    nc.sync.dma_start(out=q_sb, in_=qt_d[h])
                va_sb = va_pool.tile([128, KT, D + 1], bf16, tag="va")
                nc.sync.dma_start(out=va_sb, in_=va_d[h])
                return q_sb, k_sb, va_sb

            def mm2_closures(h, qc, wt):
                """Flat list of closures: 64 mm2 matmuls + 4 epilogues.

                Output for the whole phase (4 q-tiles) collects in one SBUF
                tile and ships as a single 256KB store (big descriptors,
                fewer completion waits on the tail).
                """
                _, _, va_sb = head_state[h]
                shared = {}
                out = []
                for qi in range(4):
                    po_box = {}

                    def mk_mm(kt, qi=qi, po_box=po_box):
                        def go():
                            if kt == 0:
                                po = out_ps_pool.tile([128, D + 1], f32, tag="po")
                                po_box["po"] = po
                            nc.tensor.matmul(
                                out=po_box["po"],
                                lhsT=wt[:, kt, qi * 128 : (qi + 1) * 128],
                                rhs=va_sb[:, kt, :],
                                start=(kt == 0),
                                stop=(kt == KT - 1),
                            )
                        return go

                    def mk_epi(qi=qi, po_box=po_box):
                        def go():
                            po = po_box["po"]
                            if qi == 0:
                                o_sb = osb_pool.tile([128, 4, D], f32, tag="osb")
                                shared["o_sb"] = o_sb
                            rc = rc_pool.tile([128, 1], f32, tag="rc")
                            nc.vector.reciprocal(out=rc, in_=po[:, D : D + 1])
                            nc.vector.tensor_scalar_mul(
                                shared["o_sb"][:, qi, :], po[:, 0:D], rc
                            )
                            if qi == 3:
                                # Last head's stores on the idle HWDGE queue.
                                store_eng = nc.sync if h == hpc - 1 else nc.gpsimd
                                store_eng.dma_start(
                                    out=o_d[h, :, qc * 4 : (qc + 1) * 4, :],
                                    in_=shared["o_sb"],
                                )
                        return go

                    for kt in range(KT):
                        out.append(mk_mm(kt))
                    out.append(mk_epi())
                return out

            def emit_phase(h, qc, fillers):
                """mm1 + exp for (h, qc); interleave filler closures."""
                if (h, qc) == (0, 0):
                    head_state[0] = load_head(0)
                if qc == QC - 1 and h + 1 < hpc:
                    head_state[h + 1] = load_head(h + 1)
                q_sb, k_sb, _ = head_state[h]
                wt = wt_pool.tile([128, KT, 512], bf16, tag="wt")
                q0 = qc * 512

                fill_iter = iter(fillers)
                for ui, (kt0, nb) in enumerate(plan):
                    ps = score_pool.tile([128, STITCH, BLK], f32, tag="score")
                    for j in range(nb):
                        kt = kt0 + j
                        nc.tensor.matmul(
                            out=ps[:, j, :],
                            lhsT=k_sb[:, kt * 128 : (kt + 1) * 128],
                            rhs=q_sb[:, q0 : q0 + 512],
                            start=True,
                            stop=True,
                        )
                    nc.scalar.activation(
                        out=wt[:, kt0 : kt0 + nb, :],
                        in_=ps[:, 0:nb, :],
                        func=mybir.ActivationFunctionType.Exp,
                        scale=SCALE,
                    )
                    # Interleave mm2 of the previous phase so the PE stream
                    # has matmul work while ACT drains the exp.
                    for _ in range(quotas[ui]):
                        nxt = next(fill_iter, None)
                        if nxt is None:
                            break
                        nxt()
                for nxt in fill_iter:
                    nxt()
                return wt

            prev = None  # (h, qc, wt) awaiting mm2
            for h in range(hpc):
                for qc in range(QC):
                    fillers = mm2_closures(*prev) if prev is not None else []
                    wt = emit_phase(h, qc, fillers)
                    prev = (h, qc, wt)
            for cl in mm2_closures(*prev):
                cl()

    return nc


def _shard_host(Q, K, V, hpc, n_cores):
    """Host-side shard + layout + cast: returns per-core input maps."""
    bf16 = ml_dtypes.bfloat16
    BH = Q.shape[0] * Q.shape[1]
    s, d = Q.shape[2], Q.shape[3]
    kt_n = s // 128
    Qf = Q.reshape(BH, s, d)
    Kf = K.reshape(BH, s, d)
    Vf = V.reshape(BH, s, d)
    # Partition-major [h, p, kt, d+1]: per-partition DMA runs are contiguous.
    Va = np.empty((BH, 128, kt_n, d + 1), dtype=bf16)
    Va[:, :, :, 0:d] = Vf.reshape(BH, kt_n, 128, d).transpose(0, 2, 1, 3).astype(bf16)
    Va[:, :, :, d] = 1.0
    in_maps = []
    for c in range(n_cores):
        sl = slice(c * hpc, (c + 1) * hpc)
        in_maps.append(
            {
                "qt": np.ascontiguousarray(
                    Qf[sl].transpose(0, 2, 1).astype(bf16)
                ),
                "kt": np.ascontiguousarray(
                    Kf[sl].transpose(0, 2, 1).astype(bf16)
                ),
                "va": Va[sl],
            }
        )
    return in_maps


def kernel(Q, K, V):
    global LAST_EXEC_TIME_NS, LAST_RESULTS
    Q = np.asarray(Q, dtype=np.float32)
    K = np.asarray(K, dtype=np.float32)
    V = np.asarray(V, dtype=np.float32)

    trace = os.environ.get("ATTN_TRACE", "0") == "1"

    key = (HPC, S)
    nc = _NC_CACHE.get(key)
    if nc is None:
        nc = build(hpc=HPC)
        nc.compile()
        _NC_CACHE[key] = nc

    in_maps = _shard_host(Q, K, V, HPC, N_CORES)
    res = run_bass_kernel_spmd(nc, in_maps, core_ids=list(range(N_CORES)), trace=trace)
    LAST_EXEC_TIME_NS = res.exec_time_ns
    LAST_RESULTS = res

    # Device out layout is partition-major [hpc, p, qt, d] -> [hpc, S, D].
    out = np.concatenate([res.results[c]["out"] for c in range(N_CORES)], axis=0)
    out = out.reshape(B * H, 128, KT, D).transpose(0, 2, 1, 3)
    return np.ascontiguousarray(out.reshape(B, H, S, D))


# revision 26
# speedup vs baseline: 1.0730x; 1.0063x over previous
"""Multi-head attention (unstabilized softmax) on 8 TRN2 NeuronCores.

Reference computes, per (batch, head):
    scores  = Q @ K^T / sqrt(d)          [S, S]
    weights = exp(scores) / rowsum(exp)  (unstabilized softmax)
    out     = weights @ V                [S, d]

Sharding: B*H = 64 (batch, head) pairs split across 8 cores -> 8 heads per
core, fully independent (no collectives).

Device pipeline per head (S=2048, d=128), q-chunk-major:
  For each q-chunk qc (512 q columns), for each k-tile kt (128 rows):
    mm1 block: scoresT[kt, qc] = K_chunk^T.T @ Q^T[:, qc]  -> PSUM [128, 512]
  Blocks land in 2 ping-ponged PSUM slots of [128, 3, 512] (3 banks each);
  ACT exp consumes 3 blocks per instruction (N=1536; plus one N=512
  remainder per phase) -> bf16 W^T chunks in SBUF.  Larger ACT tiles
  amortize the ~180-cycle per-instruction overhead (ACT is the critical
  engine: exp throughput is 1 elem/lane/cycle @ 1.2 GHz).
  mm2 (interleaved as PE filler): per 128-row q-tile, accumulate over kt:
    po[q,0:129] += W^T[kt,q].T @ [V|1][kt]   (ones col -> rowsum)
  epilogue: DVE reciprocal of col 128, per-partition scalar multiply,
  DMA out f32.  mm2 for q-chunk qc runs during the exp phase of qc+1, so
  the pipeline tail is only the final q-chunk's mm2 (~4us vs ~19us for
  head-major ordering).

Host prep: Q^T/K^T layouts [d, S] in bf16, V augmented with a ones column
([S, 129] bf16) so no device-side casts are needed.
"""

import math
import os

import numpy as np
import ml_dtypes

import concourse.bass as bass  # noqa: F401  (bass types used via APs)
import concourse.mybir as mybir
from concourse import bacc
from concourse.tile import TileContext
from concourse.bass_utils import run_bass_kernel_spmd

B, H, S, D = 4, 16, 2048, 128
N_CORES = 8
HPC = (B * H) // N_CORES  # heads per core
SCALE = 1.0 / math.sqrt(D)

KT = S // 128          # 16 k-tiles per head
QC = S // 512          # 4 q-chunks per head
BLK = 512              # score block = one mm1 matmul (N=512, one PSUM bank)
STITCH = 3             # blocks per exp instruction (3 banks per PSUM slot)

LAST_EXEC_TIME_NS = None
LAST_RESULTS = None
_NC_CACHE = {}


def build(hpc=HPC):
    f32 = mybir.dt.float32
    bf16 = mybir.dt.bfloat16

    nc = bacc.Bacc(None, target_bir_lowering=False)

    # va/out use partition-major DRAM layouts so each partition's DMA run is
    # contiguous (big descriptors; <64KB strided transfers are
    # descriptor-dominated on the SDMA engines).
    qt_d = nc.declare_dram_parameter("qt", [hpc, D, S], bf16, isOutput=False)
    kt_d = nc.declare_dram_parameter("kt", [hpc, D, S], bf16, isOutput=False)
    va_d = nc.declare_dram_parameter("va", [hpc, 128, KT, D + 1], bf16, isOutput=False)
    o_d = nc.declare_dram_parameter("out", [hpc, 128, KT, D], f32, isOutput=True)

    # exp stitch plan per phase: (kt_start, n_blocks).  The trailing units
    # are 2-blocks each (not 3+1): every exp must be longer than the next
    # unit's mm1 refill (3 blocks = ~650ns) or ACT bubbles at the phase seam.
    plan = [(0, 3), (3, 3), (6, 3), (9, 3), (12, 2), (14, 2)]
    # Unit 6's fillers sit right before the next phase's 3-block mm1 refill,
    # which must fit inside u6's 1004ns exp: 7 fillers overran it by ~135ns
    # every seam.  Shift one filler to u5 (which has 2-block refill slack).
    quotas = [13, 13, 13, 13, 10, 6]

    with TileContext(nc) as tc:
        with (
            tc.tile_pool(name="qk", bufs=2) as qk_pool,
            tc.tile_pool(name="va", bufs=2) as va_pool,
            tc.tile_pool(name="wt", bufs=3) as wt_pool,
            tc.tile_pool(name="osb", bufs=4) as osb_pool,
            tc.tile_pool(name="scoreps", bufs=2, space="PSUM") as score_pool,
            tc.tile_pool(name="outps", bufs=2, space="PSUM") as out_ps_pool,
        ):
            head_state = {}

            def load_head(h):
                """DMA head h inputs (bf16, no casts needed)."""
                q_sb = qk_pool.tile([128, S], bf16, tag="q")
                k_sb = qk_pool.tile([128, S], bf16, tag="k")
                if h == 0:
                    # The first exp needs only K cols 0:384 and Q cols
                    # 0:512: issue those two first, on separate DGE rings
                    # (each dma_start is ~600ns of serial sequencer issue).
                    nc.sync.dma_start(out=k_sb[:, 0:512], in_=kt_d[h, :, 0:512])
                    nc.gpsimd.dma_start(out=q_sb[:, 0:512], in_=qt_d[h, :, 0:512])
                    nc.sync.dma_start(out=k_sb[:, 512:S], in_=kt_d[h, :, 512:S])
                    nc.gpsimd.dma_start(out=q_sb[:, 512:S], in_=qt_d[h, :, 512:S])
                else:
                    nc.sync.dma_start(out=k_sb, in_=kt_d[h])
                    nc.gpsimd.dma_start(out=q_sb, in_=qt_d[h])
                va_sb = va_pool.tile([128, KT, D + 1], bf16, tag="va")
                nc.sync.dma_start(out=va_sb, in_=va_d[h])
                return q_sb, k_sb, va_sb

            def mm2_closures(h, qc, wt):
                """Flat list of closures: 64 mm2 matmuls + 4 epilogues.

                Output for the whole phase (4 q-tiles) collects in one SBUF
                tile and ships as a single 256KB store (big descriptors,
                fewer completion waits on the tail).
                """
                _, _, va_sb = head_state[h]
                shared = {}
                out = []
                for qi in range(4):
                    po_box = {}

                    def mk_mm(kt, qi=qi, po_box=po_box):
                        def go():
                            if kt == 0:
                                po = out_ps_pool.tile([128, D + 1], f32, tag="po")
                                po_box["po"] = po
                            nc.tensor.matmul(
                                out=po_box["po"],
                                lhsT=wt[:, kt, qi * 128 : (qi + 1) * 128],
                                rhs=va_sb[:, kt, :],
                                start=(kt == 0),
                                stop=(kt == KT - 1),
                            )
                        return go

                    def mk_epi(qi=qi, po_box=po_box):
                        def go():
                            po = po_box["po"]
                            if qi == 0:
                                o_sb = osb_pool.tile([128, 4, D], f32, tag="osb")
                                shared["o_sb"] = o_sb
                            # In-place reciprocal of the rowsum column, then
                            # one scalar multiply (no separate rc staging).
                            nc.vector.reciprocal(
                                out=po[:, D : D + 1], in_=po[:, D : D + 1]
                            )
                            nc.vector.tensor_scalar_mul(
                                shared["o_sb"][:, qi, :], po[:, 0:D], po[:, D : D + 1]
                            )
                            if qi == 3:
                                # Last head's stores on the idle HWDGE queue.
                                store_eng = nc.sync if h == hpc - 1 else nc.gpsimd
                                store_eng.dma_start(
                                    out=o_d[h, :, qc * 4 : (qc + 1) * 4, :],
                                    in_=shared["o_sb"],
                                )
                        return go

                    for kt in range(KT):
                        out.append(mk_mm(kt))
                    out.append(mk_epi())
                return out

            def emit_phase(h, qc, fillers):
                """mm1 + exp for (h, qc); interleave filler closures."""
                if (h, qc) == (0, 0):
                    head_state[0] = load_head(0)
                if qc == QC - 1 and h + 1 < hpc:
                    head_state[h + 1] = load_head(h + 1)
                q_sb, k_sb, _ = head_state[h]
                wt = wt_pool.tile([128, KT, 512], bf16, tag="wt")
                q0 = qc * 512

                fill_iter = iter(fillers)
                for ui, (kt0, nb) in enumerate(plan):
                    ps = score_pool.tile([128, STITCH, BLK], f32, tag="score")
                    for j in range(nb):
                        kt = kt0 + j
                        nc.tensor.matmul(
                            out=ps[:, j, :],
                            lhsT=k_sb[:, kt * 128 : (kt + 1) * 128],
                            rhs=q_sb[:, q0 : q0 + 512],
                            start=True,
                            stop=True,
                        )
                    nc.scalar.activation(
                        out=wt[:, kt0 : kt0 + nb, :],
                        in_=ps[:, 0:nb, :],
                        func=mybir.ActivationFunctionType.Exp,
                        scale=SCALE,
                    )
                    # Interleave mm2 of the previous phase so the PE stream
                    # has matmul work while ACT drains the exp.
                    for _ in range(quotas[ui]):
                        nxt = next(fill_iter, None)
                        if nxt is None:
                            break
                        nxt()
                for nxt in fill_iter:
                    nxt()
                return wt

            prev = None  # (h, qc, wt) awaiting mm2
            for h in range(hpc):
                for qc in range(QC):
                    fillers = mm2_closures(*prev) if prev is not None else []
                    wt = emit_phase(h, qc, fillers)
                    prev = (h, qc, wt)
            for cl in mm2_closures(*prev):
                cl()

    return nc


def _shard_host(Q, K, V, hpc, n_cores):
    """Host-side shard + layout + cast: returns per-core input maps."""
    bf16 = ml_dtypes.bfloat16
    BH = Q.shape[0] * Q.shape[1]
    s, d = Q.shape[2], Q.shape[3]
    kt_n = s // 128
    Qf = Q.reshape(BH, s, d)
    Kf = K.reshape(BH, s, d)
    Vf = V.reshape(BH, s, d)
    # Partition-major [h, p, kt, d+1]: per-partition DMA runs are contiguous.
    Va = np.empty((BH, 128, kt_n, d + 1), dtype=bf16)
    Va[:, :, :, 0:d] = Vf.reshape(BH, kt_n, 128, d).transpose(0, 2, 1, 3).astype(bf16)
    Va[:, :, :, d] = 1.0
    in_maps = []
    for c in range(n_cores):
        sl = slice(c * hpc, (c + 1) * hpc)
        in_maps.append(
            {
                "qt": np.ascontiguousarray(
                    Qf[sl].transpose(0, 2, 1).astype(bf16)
                ),
                "kt": np.ascontiguousarray(
                    Kf[sl].transpose(0, 2, 1).astype(bf16)
                ),
                "va": Va[sl],
            }
        )
    return in_maps


def kernel(Q, K, V):
    global LAST_EXEC_TIME_NS, LAST_RESULTS
    Q = np.asarray(Q, dtype=np.float32)
    K = np.asarray(K, dtype=np.float32)
    V = np.asarray(V, dtype=np.float32)

    trace = os.environ.get("ATTN_TRACE", "0") == "1"

    key = (HPC, S)
    nc = _NC_CACHE.get(key)
    if nc is None:
        nc = build(hpc=HPC)
        nc.compile()
        _NC_CACHE[key] = nc

    in_maps = _shard_host(Q, K, V, HPC, N_CORES)
    res = run_bass_kernel_spmd(nc, in_maps, core_ids=list(range(N_CORES)), trace=trace)
    LAST_EXEC_TIME_NS = res.exec_time_ns
    LAST_RESULTS = res

    # Device out layout is partition-major [hpc, p, qt, d] -> [hpc, S, D].
    out = np.concatenate([res.results[c]["out"] for c in range(N_CORES)], axis=0)
    out = out.reshape(B * H, 128, KT, D).transpose(0, 2, 1, 3)
    return np.ascontiguousarray(out.reshape(B, H, S, D))


# revision 27
# speedup vs baseline: 1.1028x; 1.0278x over previous
"""Multi-head attention (unstabilized softmax) on 8 TRN2 NeuronCores.

Reference computes, per (batch, head):
    scores  = Q @ K^T / sqrt(d)          [S, S]
    weights = exp(scores) / rowsum(exp)  (unstabilized softmax)
    out     = weights @ V                [S, d]

Sharding: B*H = 64 (batch, head) pairs split across 8 cores -> 8 heads per
core, fully independent (no collectives).

Device pipeline per head (S=2048, d=128), q-chunk-major:
  For each q-chunk qc (512 q columns), for each k-tile kt (128 rows):
    mm1 block: scoresT[kt, qc] = K_chunk^T.T @ Q^T[:, qc]  -> PSUM [128, 512]
  Blocks land in 2 ping-ponged PSUM slots of [128, 3, 512] (3 banks each);
  ACT exp consumes 3 blocks per instruction (N=1536; plus one N=512
  remainder per phase) -> bf16 W^T chunks in SBUF.  Larger ACT tiles
  amortize the ~180-cycle per-instruction overhead (ACT is the critical
  engine: exp throughput is 1 elem/lane/cycle @ 1.2 GHz).
  mm2 (interleaved as PE filler): per 128-row q-tile, accumulate over kt:
    po[q,0:129] += W^T[kt,q].T @ [V|1][kt]   (ones col -> rowsum)
  epilogue: DVE reciprocal of col 128, per-partition scalar multiply,
  DMA out f32.  mm2 for q-chunk qc runs during the exp phase of qc+1, so
  the pipeline tail is only the final q-chunk's mm2 (~4us vs ~19us for
  head-major ordering).

Host prep: Q^T/K^T layouts [d, S] in bf16, V augmented with a ones column
([S, 129] bf16) so no device-side casts are needed.
"""

import math
import os

import numpy as np
import ml_dtypes

import concourse.bass as bass  # noqa: F401  (bass types used via APs)
import concourse.mybir as mybir
from concourse import bacc
from concourse.tile import TileContext
from concourse.bass_utils import run_bass_kernel_spmd

B, H, S, D = 4, 16, 2048, 128
N_CORES = 8
HPC = (B * H) // N_CORES  # heads per core
SCALE = 1.0 / math.sqrt(D)

KT = S // 128          # 16 k-tiles per head
QC = S // 512          # 4 q-chunks per head
BLK = 512              # score block = one mm1 matmul (N=512, one PSUM bank)
STITCH = 3             # blocks per exp instruction (3 banks per PSUM slot)

LAST_EXEC_TIME_NS = None
LAST_RESULTS = None
_NC_CACHE = {}


def build(hpc=HPC):
    f32 = mybir.dt.float32
    bf16 = mybir.dt.bfloat16

    nc = bacc.Bacc(None, target_bir_lowering=False)

    # va/out use partition-major DRAM layouts so each partition's DMA run is
    # contiguous (big descriptors; <64KB strided transfers are
    # descriptor-dominated on the SDMA engines).
    qt_d = nc.declare_dram_parameter("qt", [hpc, D, S], bf16, isOutput=False)
    kt_d = nc.declare_dram_parameter("kt", [hpc, D, S], bf16, isOutput=False)
    va_d = nc.declare_dram_parameter("va", [hpc, 128, KT, D + 1], bf16, isOutput=False)
    o_d = nc.declare_dram_parameter("out", [hpc, 128, KT, D], f32, isOutput=True)

    # exp stitch plan per phase: (kt_start, n_blocks).  Each exp(u) window
    # must cover its filler quota plus the NEXT unit's mm1 refill (3 blocks
    # = ~650ns, 2 blocks = ~430ns).  Placing the short 1004ns exps (2-block
    # units) FIRST and LAST means each is followed by a cheap 2-block
    # refill, which raises total per-phase filler capacity above the 68
    # needed (the (3,3,3,3,2,2) order was ~100-140ns short in both 2-block
    # windows, gapping ACT ~240ns per phase).
    plan = [(0, 2), (2, 3), (5, 3), (8, 3), (11, 3), (14, 2)]
    quotas = [5, 13, 13, 13, 16, 8]

    with TileContext(nc) as tc:
        with (
            tc.tile_pool(name="qk", bufs=2) as qk_pool,
            tc.tile_pool(name="va", bufs=2) as va_pool,
            tc.tile_pool(name="wt", bufs=3) as wt_pool,
            tc.tile_pool(name="osb", bufs=4) as osb_pool,
            tc.tile_pool(name="scoreps", bufs=2, space="PSUM") as score_pool,
            tc.tile_pool(name="outps", bufs=2, space="PSUM") as out_ps_pool,
        ):
            head_state = {}

            def load_head(h):
                """DMA head h inputs (bf16, no casts needed)."""
                q_sb = qk_pool.tile([128, S], bf16, tag="q")
                k_sb = qk_pool.tile([128, S], bf16, tag="k")
                if h == 0:
                    # The first exp needs only K cols 0:384 and Q cols
                    # 0:512: issue those two first, on separate DGE rings
                    # (each dma_start is ~600ns of serial sequencer issue).
                    nc.sync.dma_start(out=k_sb[:, 0:512], in_=kt_d[h, :, 0:512])
                    nc.gpsimd.dma_start(out=q_sb[:, 0:512], in_=qt_d[h, :, 0:512])
                    nc.sync.dma_start(out=k_sb[:, 512:S], in_=kt_d[h, :, 512:S])
                    nc.gpsimd.dma_start(out=q_sb[:, 512:S], in_=qt_d[h, :, 512:S])
                else:
                    nc.sync.dma_start(out=k_sb, in_=kt_d[h])
                    nc.gpsimd.dma_start(out=q_sb, in_=qt_d[h])
                va_sb = va_pool.tile([128, KT, D + 1], bf16, tag="va")
                nc.sync.dma_start(out=va_sb, in_=va_d[h])
                return q_sb, k_sb, va_sb

            def mm2_closures(h, qc, wt):
                """Flat list of closures: 64 mm2 matmuls + 4 epilogues.

                Output for the whole phase (4 q-tiles) collects in one SBUF
                tile and ships as a single 256KB store (big descriptors,
                fewer completion waits on the tail).
                """
                _, _, va_sb = head_state[h]
                shared = {}
                out = []
                for qi in range(4):
                    po_box = {}

                    def mk_mm(kt, qi=qi, po_box=po_box):
                        def go():
                            if kt == 0:
                                po = out_ps_pool.tile([128, D + 1], f32, tag="po")
                                po_box["po"] = po
                            nc.tensor.matmul(
                                out=po_box["po"],
                                lhsT=wt[:, kt, qi * 128 : (qi + 1) * 128],
                                rhs=va_sb[:, kt, :],
                                start=(kt == 0),
                                stop=(kt == KT - 1),
                            )
                        return go

                    def mk_epi(qi=qi, po_box=po_box):
                        def go():
                            po = po_box["po"]
                            if qi == 0:
                                o_sb = osb_pool.tile([128, 4, D], f32, tag="osb")
                                shared["o_sb"] = o_sb
                            # In-place reciprocal of the rowsum column, then
                            # one scalar multiply (no separate rc staging).
                            nc.vector.reciprocal(
                                out=po[:, D : D + 1], in_=po[:, D : D + 1]
                            )
                            nc.vector.tensor_scalar_mul(
                                shared["o_sb"][:, qi, :], po[:, 0:D], po[:, D : D + 1]
                            )
                            if qi == 3:
                                # Last head's stores on the idle HWDGE queue.
                                store_eng = nc.sync if h == hpc - 1 else nc.gpsimd
                                store_eng.dma_start(
                                    out=o_d[h, :, qc * 4 : (qc + 1) * 4, :],
                                    in_=shared["o_sb"],
                                )
                        return go

                    for kt in range(KT):
                        out.append(mk_mm(kt))
                    out.append(mk_epi())
                return out

            def emit_phase(h, qc, fillers):
                """mm1 + exp for (h, qc); interleave filler closures."""
                if (h, qc) == (0, 0):
                    head_state[0] = load_head(0)
                if qc == QC - 1 and h + 1 < hpc:
                    head_state[h + 1] = load_head(h + 1)
                q_sb, k_sb, _ = head_state[h]
                wt = wt_pool.tile([128, KT, 512], bf16, tag="wt")
                q0 = qc * 512

                fill_iter = iter(fillers)
                for ui, (kt0, nb) in enumerate(plan):
                    ps = score_pool.tile([128, STITCH, BLK], f32, tag="score")
                    for j in range(nb):
                        kt = kt0 + j
                        nc.tensor.matmul(
                            out=ps[:, j, :],
                            lhsT=k_sb[:, kt * 128 : (kt + 1) * 128],
                            rhs=q_sb[:, q0 : q0 + 512],
                            start=True,
                            stop=True,
                        )
                    nc.scalar.activation(
                        out=wt[:, kt0 : kt0 + nb, :],
                        in_=ps[:, 0:nb, :],
                        func=mybir.ActivationFunctionType.Exp,
                        scale=SCALE,
                    )
                    # Interleave mm2 of the previous phase so the PE stream
                    # has matmul work while ACT drains the exp.
                    for _ in range(quotas[ui]):
                        nxt = next(fill_iter, None)
                        if nxt is None:
                            break
                        nxt()
                for nxt in fill_iter:
                    nxt()
                return wt

            prev = None  # (h, qc, wt) awaiting mm2
            for h in range(hpc):
                for qc in range(QC):
                    fillers = mm2_closures(*prev) if prev is not None else []
                    wt = emit_phase(h, qc, fillers)
                    prev = (h, qc, wt)
            for cl in mm2_closures(*prev):
                cl()

    return nc


def _shard_host(Q, K, V, hpc, n_cores):
    """Host-side shard + layout + cast: returns per-core input maps."""
    bf16 = ml_dtypes.bfloat16
    BH = Q.shape[0] * Q.shape[1]
    s, d = Q.shape[2], Q.shape[3]
    kt_n = s // 128
    Qf = Q.reshape(BH, s, d)
    Kf = K.reshape(BH, s, d)
    Vf = V.reshape(BH, s, d)
    # Partition-major [h, p, kt, d+1]: per-partition DMA runs are contiguous.
    Va = np.empty((BH, 128, kt_n, d + 1), dtype=bf16)
    Va[:, :, :, 0:d] = Vf.reshape(BH, kt_n, 128, d).transpose(0, 2, 1, 3).astype(bf16)
    Va[:, :, :, d] = 1.0
    in_maps = []
    for c in range(n_cores):
        sl = slice(c * hpc, (c + 1) * hpc)
        in_maps.append(
            {
                "qt": np.ascontiguousarray(
                    Qf[sl].transpose(0, 2, 1).astype(bf16)
                ),
                "kt": np.ascontiguousarray(
                    Kf[sl].transpose(0, 2, 1).astype(bf16)
                ),
                "va": Va[sl],
            }
        )
    return in_maps


def kernel(Q, K, V):
    global LAST_EXEC_TIME_NS, LAST_RESULTS
    Q = np.asarray(Q, dtype=np.float32)
    K = np.asarray(K, dtype=np.float32)
    V = np.asarray(V, dtype=np.float32)

    trace = os.environ.get("ATTN_TRACE", "0") == "1"

    key = (HPC, S)
    nc = _NC_CACHE.get(key)
    if nc is None:
        nc = build(hpc=HPC)
        nc.compile()
        _NC_CACHE[key] = nc

    in_maps = _shard_host(Q, K, V, HPC, N_CORES)
    res = run_bass_kernel_spmd(nc, in_maps, core_ids=list(range(N_CORES)), trace=trace)
    LAST_EXEC_TIME_NS = res.exec_time_ns
    LAST_RESULTS = res

    # Device out layout is partition-major [hpc, p, qt, d] -> [hpc, S, D].
    out = np.concatenate([res.results[c]["out"] for c in range(N_CORES)], axis=0)
    out = out.reshape(B * H, 128, KT, D).transpose(0, 2, 1, 3)
    return np.ascontiguousarray(out.reshape(B, H, S, D))
